# revision 13
# baseline (speedup 1.0000x reference)
"""GAT layer (gnn_message_passing) on 8 Trainium2 NeuronCores via Bass/Tile.

Strategy (edge-parallel, node-sharded output):
  * Host: sort each core's edges (rows sharded 12500/core) by destination
    row into 98 node-windows of 128 rows; within a window, group edges by
    source-node bucket (4 buckets of ~25K nodes so gather indices fit
    int16); pad each (window, bucket) group to a whole number of 128-edge
    tiles.  The per-(w,b) tile counts are maxed across cores so all 8
    cores run one identical SPMD program.
  * Phase 1 (device, replicated): feature table ztab[phys, 128] fp16 =
    x @ W^T in depth-major feature order, stored partition-interleaved
    (node n at phys row (n%128)*NT + n//128) so table writes coalesce.
  * Phase 2: per 128-edge tile, DMAGather h[col] (256B rows);
    w = max(exp(a1), exp(a2)) with a1 = al+ar+ln(pos), a2 = 0.2*(al+ar)
    +ln(pos) streamed per edge from the host (LeakyReLU folded into the
    max-of-exps since exp is monotone); rhs = [w * h[col] | w]; one-hot
    scatter matmul S^T @ rhs accumulates [sum w*h | sum w] per window in
    PSUM; per window, scale by 1/max(sum w, tiny) and DMA the 128-node
    output slice.
"""
import numpy as np

import concourse.bass as bass
import concourse.mybir as mybir
import concourse.tile as tile
from concourse import bacc
from concourse.bass_utils import run_bass_kernel_spmd

F32 = mybir.dt.float32
F16 = mybir.dt.float16
I16 = mybir.dt.int16

GEO = dict(N=100000, E=1600000, IN=128, HEADS=8, HID=16, NCORES=8)
P = 128
NBK = 4            # gather-index buckets (int16 limit)
SPAN = 4           # windows per gather batch


def host_prep(x, edge_index, pos, W, att, geo=GEO):
    N, IN, HEADS, HID, NC = geo["N"], geo["IN"], geo["HEADS"], geo["HID"], geo["NCORES"]
    NPC = N // NC
    NW = (NPC + P - 1) // P
    NT = (N + P - 1) // P                       # phase-1 node tiles
    NPHYS = NT * P
    BSZ = (NPHYS + NBK - 1) // NBK              # phys rows per bucket
    assert BSZ <= 32768

    dm_of_std = np.arange(HEADS * HID).reshape(HEADS, HID).T.reshape(-1)
    W_dm = W[dm_of_std]
    vl = (W.reshape(HEADS, HID, IN) * att[0, :, :HID][:, :, None]).sum(1).T
    vr = (W.reshape(HEADS, HID, IN) * att[0, :, HID:][:, :, None]).sum(1).T
    lpos = np.log(pos.astype(np.float64)).astype(np.float32)

    # host-side attention logits (cheap: 2 x [N,128]@[128,8])
    al = (x @ vl).astype(np.float32)            # [N, 8]
    ar = (x @ vr).astype(np.float32)

    row = edge_index[0].astype(np.int64)
    col = edge_index[1].astype(np.int64)
    phys = (col % P) * NT + col // P            # physical ztab row of col
    bkt = phys // BSZ

    # per (core, window, bucket) edge lists
    core = row // NPC
    TWB = np.zeros((NW, NBK), np.int64)         # tiles per (w,b), maxed over cores
    groups = []                                 # [core][w][b] -> (cols_phys_loc, a1, a2, rw)
    for k in range(NC):
        m = core == k
        r_k, c_k, ph_k, bk_k = row[m], col[m], phys[m], bkt[m]
        lr = r_k - k * NPC
        w_k = lr // P
        order = np.lexsort((bk_k, w_k))
        r_k, c_k, ph_k, bk_k, lr, w_k = (a[order] for a in (r_k, c_k, ph_k, bk_k, lr, w_k))
        a1 = (al[r_k] + ar[c_k] + lpos[c_k][:, None]).astype(np.float16)
        a2 = (0.2 * (al[r_k] + ar[c_k]) + lpos[c_k][:, None]).astype(np.float16)
        rwv = (lr % P).astype(np.float32)
        loc = (ph_k - bk_k * BSZ).astype(np.int16)
        gw = []
        key = w_k * NBK + bk_k
        cnt = np.bincount(key, minlength=NW * NBK).reshape(NW, NBK)
        np.maximum(TWB, (cnt + P - 1) // P, out=TWB)
        off = 0
        for wi in range(NW):
            gb = []
            for b in range(NBK):
                n = int(cnt[wi, b])
                gb.append((loc[off:off + n], a1[off:off + n], a2[off:off + n],
                           rwv[off:off + n]))
                off += n
            gw.append(gb)
        groups.append(gw)
    TWB = TWB.astype(np.int64)
    totT = int(TWB.sum())

    # build padded per-core streams in (w, b) tile order
    # edge j of a group -> slot j//128, partition j%128
    eidx_s = np.zeros((NC, P, totT * 8), np.int16)    # 16-wrap idx, repl x8
    a12_s = np.full((NC, P, totT * 16), -60000.0, np.float16)
    rw_s = np.zeros((NC, P, totT), np.float32)
    tile_off = np.zeros((NW, NBK), np.int64)
    o = 0
    for wi in range(NW):
        for b in range(NBK):
            tile_off[wi, b] = o
            o += TWB[wi, b]
    for k in range(NC):
        for wi in range(NW):
            for b in range(NBK):
                loc, a1, a2, rwv = groups[k][wi][b]
                n = len(loc)
                t0 = tile_off[wi, b]
                npad = int(TWB[wi, b]) * P
                li = np.zeros(npad, np.int16)
                li[:n] = loc
                # idx layout for the (w,b) gather: j -> [j%16, j//16] repl x8
                i16 = li.reshape(-1, 16).T          # [16, npad/16]
                eidx_s[k, :, t0 * 8:(t0 + int(TWB[wi, b])) * 8] = np.tile(
                    i16, (8, 1))
                jj = np.arange(n)
                pj, tj = jj % P, t0 + jj // P
                a_blk = np.full((P, int(TWB[wi, b]), 16), -60000.0, np.float16)
                a_blk[pj, tj - t0, :8] = a1
                a_blk[pj, tj - t0, 8:] = a2
                a12_s[k, :, t0 * 16:(t0 + int(TWB[wi, b])) * 16] = \
                    a_blk.reshape(P, -1)
                r_blk = np.zeros((P, int(TWB[wi, b])), np.float32)
                r_blk[pj, tj - t0] = rwv
                rw_s[k, :, t0:t0 + int(TWB[wi, b])] = r_blk

    # phase-1 constants
    lposT = np.zeros((P, NT), np.float32)
    consts = np.zeros((P, 2 * P), np.float16)
    consts[:, :P] = np.eye(P, dtype=np.float16)
    consts[:, P:2 * P] = np.arange(P)[None, :].astype(np.float16)

    ins = dict(
        xT=np.ascontiguousarray(x.T.astype(np.float16)),
        wdm=np.ascontiguousarray(W_dm.T.astype(np.float16)),   # [IN, 128]
        consts=consts,
    )
    in_maps = []
    for k in range(NC):
        m = dict(ins)
        m["eidx"] = np.ascontiguousarray(eidx_s[k])
        m["a12"] = np.ascontiguousarray(a12_s[k])
        m["rw"] = np.ascontiguousarray(rw_s[k])
        in_maps.append(m)
    meta = dict(TWB=TWB, totT=totT, NW=NW, NPC=NPC, NT=NT, BSZ=BSZ,
                dm_of_std=dm_of_std, geo=geo)
    return in_maps, meta


def build_program(meta):
    geo = meta["geo"]
    N, IN, NC = geo["N"], geo["IN"], geo["NCORES"]
    TWB, totT, NW, NPC, NT, BSZ = (meta[k] for k in
                                   ("TWB", "totT", "NW", "NPC", "NT", "BSZ"))
    NPHYS = NT * P
    WB = 8  # phase-1 write batch (tiles per ztab DMA)

    nc = bacc.Bacc("TRN2", target_bir_lowering=False, debug=False,
                   num_devices=NC)
    xT = nc.dram_tensor("xT", [IN, N], F16, kind="ExternalInput")
    wdm = nc.dram_tensor("wdm", [IN, IN], F16, kind="ExternalInput")
    consts = nc.dram_tensor("consts", [P, 2 * P], F16, kind="ExternalInput")
    eidx = nc.dram_tensor("eidx", [P, totT * 8], I16, kind="ExternalInput")
    a12 = nc.dram_tensor("a12", [P, totT * 16], F16, kind="ExternalInput")
    rw = nc.dram_tensor("rw", [P, totT], F32, kind="ExternalInput")
    out = nc.dram_tensor("out", [NPC, IN], F32, kind="ExternalOutput")
    ztab = nc.dram_tensor("ztab", [NPHYS, IN], F16)
    ztab_r = ztab[:, :].rearrange("(p i) f -> p i f", i=NT)

    AF = mybir.ActivationFunctionType
    OP = mybir.AluOpType

    def bc(ap, reps, inner):
        return bass.AP(ap.tensor, ap.offset,
                       [list(ap.ap[0]), [0, reps], [1, inner]])

    # static gather batching: spans of SPAN windows
    spans = []
    for w0 in range(0, NW, SPAN):
        ws = list(range(w0, min(w0 + SPAN, NW)))
        spans.append(ws)

    with tile.TileContext(nc) as tc:
        cst_cm = tc.tile_pool(name="cst", bufs=1)
        cst = cst_cm.__enter__()
        consts_sb = cst.tile([P, 2 * P], F16, tag="consts")
        nc.sync.dma_start(out=consts_sb[:], in_=consts[:, :])

        # ---------------- phase 1: ztab = h (depth-major, interleaved) ---
        with (
            tc.tile_pool(name="p1", bufs=3) as p1,
            tc.tile_pool(name="p1s", bufs=2) as p1s,
            tc.tile_pool(name="p1ps", bufs=4, space="PSUM") as p1ps,
        ):
            zwrites = []
            wdm_sb = p1s.tile([IN, IN], F16, tag="wdm")
            nc.sync.dma_start(out=wdm_sb[:], in_=wdm[:, :])
            for i0 in range(0, NT, WB):
                nb = min(WB, NT - i0)
                stg = p1s.tile([P, WB * IN], F16, tag="stg")
                for j in range(nb):
                    i = i0 + j
                    n0 = i * P
                    nn = min(P, N - n0)
                    if nn < P:
                        nc.vector.memset(stg[:, j * IN:(j + 1) * IN], 0.0)
                    xt = p1.tile([IN, P], F16, tag="xt")
                    nc.sync.dma_start(out=xt[:, :nn], in_=xT[:, n0:n0 + nn])
                    ps = p1ps.tile([P, IN], F32, tag="p1ps")
                    nc.tensor.matmul(out=ps[:nn, :], lhsT=xt[:, :nn],
                                     rhs=wdm_sb[:], start=True, stop=True)
                    # split PSUM->SBUF copy across ScalarE and VectorE
                    nc.scalar.activation(stg[:nn, j * IN:j * IN + 80],
                                         ps[:nn, 0:80], AF.Copy)
                    nc.vector.tensor_copy(stg[:nn, j * IN + 80:(j + 1) * IN],
                                          ps[:nn, 80:IN])
                zw = nc.sync.dma_start(out=ztab_r[:, i0:i0 + nb, :],
                                  in_=stg[:].rearrange("p (i f) -> p i f", f=IN)[:, :nb, :])
                zwrites.append(zw.ins)

        # ---------------- phase 2: edge processing ----------------
        zfence = None
        with (
            tc.tile_pool(name="gth", bufs=2) as gth,
            tc.tile_pool(name="stm", bufs=2) as stm,
            tc.tile_pool(name="wrk", bufs=4) as wrk,
            tc.tile_pool(name="ops", bufs=2, space="PSUM") as opsp,
            tc.tile_pool(name="fin", bufs=2) as fin,
        ):
            iota_b = consts_sb[:, P:2 * P]
            for ws in spans:
                w0, w1 = ws[0], ws[-1]
                sT = {b: int(TWB[ws, b].sum()) for b in range(NBK)}
                t0w = int(TWB[:w0].sum())                 # first tile of span
                sTot = int(TWB[ws, :].sum())
                # stream chunks for the span
                a12_sb = stm.tile([P, sTot * 16], F16, tag="a12")
                nc.sync.dma_start(out=a12_sb[:],
                                  in_=a12[:, t0w * 16:(t0w + sTot) * 16])
                rw_sb = stm.tile([P, sTot], F32, tag="rw")
                nc.sync.dma_start(out=rw_sb[:], in_=rw[:, t0w:t0w + sTot])
                # per-bucket gathers for the whole span
                gts = {}
                for b in range(NBK):
                    nb = sT[b]
                    if nb == 0:
                        continue
                    gt = gth.tile([P, max(nb, 1) * IN], F16, tag=f"gt{b}")
                    idx_sb = stm.tile([P, max(nb, 1) * 8], I16, tag=f"ei{b}")
                    # stream offset of this (span, bucket): tiles of bucket b
                    # within span start at span tile offsets; they are laid
                    # out contiguously per (w, b) group in stream order, so
                    # copy per-window slices into one SBUF run via one DMA
                    # each (w groups of bucket b are not adjacent in the
                    # global stream).
                    o = 0
                    for w in ws:
                        tw = int(TWB[w, b])
                        if tw == 0:
                            continue
                        tg = int(TWB[:w].sum() + TWB[w, :b].sum())
                        nc.sync.dma_start(
                            out=idx_sb[:, o * 8:(o + tw) * 8],
                            in_=eidx[:, tg * 8:(tg + tw) * 8])
                        o += tw
                    blo = b * BSZ
                    bhi = min(NPHYS, blo + BSZ)
                    gt3 = gt[:].rearrange("p (t f) -> p t f", f=IN)
                    GMAX = 8  # <=1024 descriptors per SWDGE gather
                    for c0 in range(0, nb, GMAX):
                        ct = min(GMAX, nb - c0)
                        nc.gpsimd.dma_gather(
                            out_ap=gt3[:, c0:c0 + ct, :],
                            in_ap=ztab[blo:bhi, :],
                            idxs_ap=idx_sb[:, c0 * 8:(c0 + ct) * 8],
                            num_idxs=ct * P,
                            num_idxs_reg=ct * P,
                            elem_size=IN,
                        )
                    gts[b] = gt

                boff = {b: 0 for b in range(NBK)}
                for w in ws:
                    nn = min(P, NPC - w * P)
                    Tw = int(TWB[w, :].sum())
                    ops = opsp.tile([P, 136], F32, tag="ops")
                    ti = 0
                    for b in range(NBK):
                        tw = int(TWB[w, b])
                        for t in range(tw):
                            gt3 = gts[b][:].rearrange("p (t f) -> p t f", f=IN)
                            zt = gt3[:, boff[b] + t, :]
                            goff = int(TWB[:w].sum() + TWB[w, :b].sum()) - t0w + t
                            S = wrk.tile([P, P], F16, tag="S")
                            nc.vector.tensor_scalar(
                                S[:], iota_b, rw_sb[:, goff:goff + 1], None,
                                OP.is_equal)
                            w1 = wrk.tile([P, 8], F16, tag="w1")
                            w2 = wrk.tile([P, 8], F16, tag="w2")
                            nc.scalar.activation(
                                w1[:], a12_sb[:, goff * 16:goff * 16 + 8], AF.Exp)
                            nc.scalar.activation(
                                w2[:], a12_sb[:, goff * 16 + 8:goff * 16 + 16],
                                AF.Exp)
                            rhs = wrk.tile([P, 136], F16, tag="rhs")
                            nc.vector.tensor_tensor(out=rhs[:, 128:136],
                                                    in0=w1[:], in1=w2[:],
                                                    op=OP.max)
                            nc.vector.tensor_tensor(
                                out=rhs[:, 0:128].rearrange(
                                    "p (a b) -> p a b", b=8),
                                in0=zt.rearrange("p (a b) -> p a b", b=8),
                                in1=bc(rhs[:, 128:136], 16, 8), op=OP.mult)
                            nc.tensor.matmul(out=ops[:], lhsT=S[:], rhs=rhs[:],
                                             start=(ti == 0),
                                             stop=(ti == Tw - 1))
                            ti += 1
                        boff[b] += tw
                    rsc = fin.tile([P, 8], F32, tag="rsc")
                    nc.vector.tensor_scalar(rsc[:], ops[:, 128:136], 1e-30,
                                            None, OP.max)
                    inv = fin.tile([P, 8], F32, tag="inv")
                    nc.vector.reciprocal(inv[:], rsc[:])
                    ostg = fin.tile([P, IN], F32, tag="ostg")
                    nc.vector.tensor_tensor(
                        out=ostg[:].rearrange("p (a b) -> p a b", b=8),
                        in0=ops[:, 0:IN].rearrange("p (a b) -> p a b", b=8),
                        in1=bc(inv[:], 16, 8), op=OP.mult)
                    nc.sync.dma_start(out=out[w * P:w * P + nn, :],
                                      in_=ostg[:nn, :])
        cst_cm.__exit__(None, None, None)
    nc.finalize()
    return nc


def kernel(x, edge_index, pos, W, att):
    x = np.asarray(x, np.float32)
    edge_index = np.asarray(edge_index, np.int32)
    pos = np.asarray(pos, np.float32)
    W = np.asarray(W, np.float32)
    att = np.asarray(att, np.float32)
    in_maps, meta = host_prep(x, edge_index, pos, W, att)
    nc = build_program(meta)
    res = run_bass_kernel_spmd(nc, in_maps, core_ids=list(range(GEO["NCORES"])))
    shards = [res.results[k]["out"] for k in range(GEO["NCORES"])]
    out_dev = np.concatenate(shards, 0)
    inv_perm = np.argsort(meta["dm_of_std"])
    return np.ascontiguousarray(out_dev[:, inv_perm]).astype(np.float32)
